# revision 11
# baseline (speedup 1.0000x reference)
"""AttentionPooling (segment softmax + weighted segment-sum) on 8 trn2 cores.

Strategy: shard nodes across cores at segment-aligned cuts (batch is sorted).
Host pre-casts x to bf16 and pre-transposes it, so the device streams both
x [n, d] (pooling values) and xT [d, n] (MLP operand) from HBM at line rate
with no on-chip transposes. Per 128-node tile: MLP scores (bf16 matmuls,
fp32 PSUM) -> exp -> scatter-matmul (A_e^T @ x) accumulating [128seg, 256+1]
in PSUM over a 124-tile window; windows chain via a carried partial row for
the straddling segment. The A_e matmuls are software-pipelined one tile-group
behind the score pipeline so the tensor engine never waits on ACT/DVE.
Normalization (U/Z) on device. No collectives; host scatters the per-window
rows into the final [4096, 256] output.
"""

import ml_dtypes
import numpy as np

BF16 = ml_dtypes.bfloat16

# ---------------------------------------------------------------- constants
N_FULL = 1_000_000
D = 256
H = 128
G = 4096
NCORES = 8
P = 128

TILES = 992                 # node tiles per core
NC_PAD = TILES * P          # 126976 padded nodes per core
WINDOWS = 8
TPW = TILES // WINDOWS      # 124 tiles per window
WIN_NODES = TPW * P         # 15872
SUPER = 31                  # node tiles per DMA super-tile
SUPERS_PER_WIN = TPW // SUPER   # 4
OUT_ROWS = WINDOWS * P      # 1024 rows per core
EPS = 1e-30


def _set_config(tiles, windows, super_):
    """Reconfigure problem tiling (testing only; defaults are production)."""
    global TILES, NC_PAD, WINDOWS, TPW, WIN_NODES, SUPER, SUPERS_PER_WIN, OUT_ROWS
    TILES, WINDOWS, SUPER = tiles, windows, super_
    NC_PAD = TILES * P
    TPW = TILES // WINDOWS
    assert TPW * WINDOWS == TILES
    WIN_NODES = TPW * P
    SUPERS_PER_WIN = TPW // super_
    assert SUPERS_PER_WIN * super_ == TPW
    OUT_ROWS = WINDOWS * P
    _NC_CACHE.clear()


# ---------------------------------------------------------------- host plan
def _plan(batch):
    """batch: sorted int array [N]. Returns per-core planning dicts."""
    batch = np.asarray(batch).astype(np.int64).ravel()
    n = batch.shape[0]
    # all segment-start positions (including 0 and n)
    change = np.flatnonzero(np.diff(batch)) + 1
    bounds = np.concatenate([[0], change, [n]])
    cuts = [0]
    for c in range(1, NCORES):
        tgt = c * n // NCORES
        i = np.searchsorted(bounds, tgt)
        lo = bounds[i - 1] if i > 0 else bounds[0]
        hi = bounds[min(i, len(bounds) - 1)]
        cut = int(lo if (tgt - lo) <= (hi - tgt) else hi)
        cuts.append(cut)
    cuts.append(n)
    for i in range(NCORES):
        assert cuts[i] < cuts[i + 1], f"empty core shard {i}: {cuts}"
        assert cuts[i + 1] - cuts[i] <= NC_PAD, (
            f"core {i} shard {cuts[i + 1] - cuts[i]} > NC_PAD={NC_PAD}"
        )

    plans = []
    for c in range(NCORES):
        lo, hi = cuts[c], cuts[c + 1]
        n_c = hi - lo
        local = batch[lo:hi]
        rel = np.full(NC_PAD, -1.0, np.float32)
        bases = np.full(WINDOWS, -1, np.int64)
        for w in range(WINDOWS):
            a = w * WIN_NODES
            b = min((w + 1) * WIN_NODES, n_c)
            if a >= n_c:
                continue
            base = int(local[a])
            bases[w] = base
            r = local[a:b] - base
            assert r.min() >= 0 and r.max() < P, (
                f"core {c} window {w}: {P} seg rows exceeded (max rel {r.max()})"
            )
            rel[a:b] = r.astype(np.float32)

        last_seg = int(local[-1])
        onehot = np.zeros((P, WINDOWS), np.float32)
        valid = []  # (global_seg_start, nrows) per window
        for w in range(WINDOWS):
            if bases[w] < 0:
                valid.append((0, 0))
                continue
            nxt = bases[w + 1] if (w + 1 < WINDOWS and bases[w + 1] >= 0) else -1
            if nxt >= 0:
                diff = int(nxt - bases[w])
                assert 0 < diff < P, f"core {c} window {w}: carry diff {diff}"
                onehot[diff, w] = 1.0
                hi_seg = nxt
            else:
                hi_seg = last_seg + 1
            nrows = hi_seg - int(bases[w])
            assert 0 < nrows <= P
            valid.append((int(bases[w]), int(nrows)))

        # rel_seg rearranged so partition p, col t = rel[t*P + p]
        rel_arr = rel.reshape(TILES, P).T.copy()  # [P, TILES]
        plans.append(
            dict(lo=lo, hi=hi, n_c=n_c, rel_arr=rel_arr, onehot=onehot, valid=valid)
        )
    return plans


def _make_in_maps(x, W1, b1, W2, b2, plans):
    x = np.asarray(x)
    xb_full = x.astype(BF16)
    W1b = np.ascontiguousarray(np.asarray(W1, np.float32)).astype(BF16)
    b1f = np.ascontiguousarray(np.asarray(b1, np.float32)).reshape(H, 1)
    W2b = np.repeat(
        np.asarray(W2, np.float32).reshape(H, 1), 2, axis=1
    ).astype(BF16)
    b2f = np.ascontiguousarray(np.asarray(b2, np.float32)).reshape(1, 1)
    nsup = TILES // SUPER
    in_maps = []
    for pl in plans:
        xp = np.zeros((NC_PAD, D), BF16)
        xp[: pl["n_c"]] = xb_full[pl["lo"] : pl["hi"]]
        xt = np.ascontiguousarray(xp.T)  # [D, NC_PAD] bf16
        # natural x in the exact SBUF super-tile layout, so each DMA reads
        # one contiguous chunk per partition: [sup, p, t*D + c]
        xr = np.ascontiguousarray(
            xp.reshape(nsup, SUPER, P, D).transpose(0, 2, 1, 3)
        ).reshape(nsup, P, SUPER * D)
        in_maps.append(
            {
                "x": xr,
                "xt": xt,
                "relseg": pl["rel_arr"].astype(BF16),
                "onehot": pl["onehot"],
                "w1": W1b,
                "b1": b1f,
                "w2": W2b,
                "b2": b2f,
            }
        )
    return in_maps


def _assemble(outs, plans, dtype):
    final = np.zeros((G, D), dtype)
    for pl, o in zip(plans, outs):
        for w, (g0, nrows) in enumerate(pl["valid"]):
            if nrows:
                final[g0 : g0 + nrows] = o[w * P : w * P + nrows]
    return final


def _super_groups():
    """Tile groups within a super-tile: eights then a remainder group."""
    gs = []
    a = 0
    while a < SUPER:
        gn = min(8, SUPER - a)
        gs.append((a, gn))
        a += gn
    return gs


# ------------------------------------------------------------ numpy emulator
def _emulate(inputs):
    """Pure-numpy emulation of the device program (for logic validation)."""
    x = np.asarray(inputs["x"], np.float32)
    W1 = np.asarray(inputs["W1"], np.float32)
    b1 = np.asarray(inputs["b1"], np.float32)
    W2 = np.asarray(inputs["W2"], np.float32)
    b2 = np.asarray(inputs["b2"], np.float32)
    plans = _plan(inputs["batch"])
    in_maps = _make_in_maps(x, W1, b1, W2, b2, plans)
    outs = []
    cols = np.arange(P, dtype=np.float32)[None, :]
    nsup = TILES // SUPER
    for im in in_maps:
        xp = (
            np.asarray(im["x"], np.float32)
            .reshape(nsup, P, SUPER, D)
            .transpose(0, 2, 1, 3)
            .reshape(NC_PAD, D)
        )
        rel = im["relseg"].T.reshape(-1)  # [NC_PAD] node order
        h = np.tanh(xp @ W1 + b1.reshape(1, H))
        s = (h @ W2[:, 0:1]).ravel() + float(b2.ravel()[0])
        e = np.exp(s)
        out = np.zeros((OUT_ROWS, D), np.float32)
        carry = np.zeros(D + 1, np.float32)
        for w in range(WINDOWS):
            uz = np.zeros((P, D + 1), np.float32)
            a, b = w * WIN_NODES, (w + 1) * WIN_NODES
            A = (cols == rel[a:b, None]).astype(np.float32) * e[a:b, None]
            uz[:, :D] = A.T @ xp[a:b]
            uz[:, D] = A.sum(axis=0)
            uz[0] += carry
            carry = im["onehot"][:, w] @ uz
            out[w * P : (w + 1) * P] = uz[:, :D] / (uz[:, D : D + 1] + EPS)
        outs.append(out)
    return _assemble(outs, plans, np.float32)


# ------------------------------------------------------------- bass program
_NC_CACHE = {}


def _build_nc():
    if "nc" in _NC_CACHE:
        return _NC_CACHE["nc"]
    import concourse.bacc as bacc
    import concourse.mybir as mybir
    import concourse.tile as tile

    f32 = mybir.dt.float32
    bf16 = mybir.dt.bfloat16
    AF = mybir.ActivationFunctionType
    ALU = mybir.AluOpType

    nc = bacc.Bacc(None, target_bir_lowering=False)

    nsup = TILES // SUPER
    x_d = nc.dram_tensor("x", [nsup, P, SUPER * D], bf16, kind="ExternalInput")
    xt_d = nc.dram_tensor("xt", [D, NC_PAD], bf16, kind="ExternalInput")
    rel_d = nc.dram_tensor("relseg", [P, TILES], bf16, kind="ExternalInput")
    oh_d = nc.dram_tensor("onehot", [P, WINDOWS], f32, kind="ExternalInput")
    w1_d = nc.dram_tensor("w1", [D, H], bf16, kind="ExternalInput")
    b1_d = nc.dram_tensor("b1", [H, 1], f32, kind="ExternalInput")
    w2_d = nc.dram_tensor("w2", [H, 2], bf16, kind="ExternalInput")
    b2_d = nc.dram_tensor("b2", [1, 1], f32, kind="ExternalInput")
    out_d = nc.dram_tensor("out", [OUT_ROWS, D], f32, kind="ExternalOutput")

    with tile.TileContext(nc) as tc:
        with (
            tc.tile_pool(name="singles", bufs=1) as singles,
            tc.tile_pool(name="xsup", bufs=3) as xpool,
            tc.tile_pool(name="xtsup", bufs=3) as xt_pool,
            tc.tile_pool(name="hb", bufs=2) as hb_pool,
            tc.tile_pool(name="e", bufs=3) as e_pool,
            tc.tile_pool(name="ae", bufs=4) as ae_pool,
            tc.tile_pool(name="flush", bufs=2) as flush_pool,
            tc.tile_pool(name="ps_h", bufs=2, space="PSUM") as ps_h,
            tc.tile_pool(name="ps_s", bufs=1, space="PSUM") as ps_s,
            tc.tile_pool(name="ps_uz", bufs=2, space="PSUM") as ps_uz,
            tc.tile_pool(name="ps_c", bufs=1, space="PSUM") as ps_c,
        ):
            iota_i = singles.tile([P, P], mybir.dt.int32)
            nc.gpsimd.iota(iota_i[:], pattern=[[1, P]], base=0, channel_multiplier=0)
            iota8 = singles.tile([P, 8, P], bf16)
            for j in range(8):
                nc.vector.tensor_copy(out=iota8[:, j, :], in_=iota_i[:])

            w1_sb = singles.tile([P, 2, H], bf16)
            w1_r = w1_d[:].rearrange("(c k) m -> c k m", c=2)
            nc.sync.dma_start(out=w1_sb[:, 0, :], in_=w1_r[0])
            nc.sync.dma_start(out=w1_sb[:, 1, :], in_=w1_r[1])
            b1_sb = singles.tile([P, 1], f32)
            nc.sync.dma_start(out=b1_sb[:], in_=b1_d[:])
            w2_sb = singles.tile([P, 2], bf16)
            nc.sync.dma_start(out=w2_sb[:], in_=w2_d[:])
            b2_sb = singles.tile([P, 1], f32)
            nc.sync.dma_start(out=b2_sb[:], in_=b2_d[:].to_broadcast([P, 1]))
            oh_sb = singles.tile([P, WINDOWS], f32)
            nc.sync.dma_start(out=oh_sb[:], in_=oh_d[:])
            rel_sb = singles.tile([P, TILES], bf16)
            nc.sync.dma_start(out=rel_sb[:], in_=rel_d[:])
            ones_sb = singles.tile([P, 1], bf16)
            nc.vector.memset(ones_sb[:], 1.0)
            carry_sb = singles.tile([1, D + 1], f32)
            nc.vector.memset(carry_sb[:], 0.0)

            x_r = x_d[:].rearrange("s p (t c) -> s p t c", t=SUPER)
            xt_r = xt_d[:].rearrange("(k p) n -> k p n", k=2)
            gs = _super_groups()

            for w in range(WINDOWS):
                uz_ps = ps_uz.tile([P, D + 2], f32)

                def emit_ae(sup_t, sg_, a_, gn_, e_sb_, w=w, uz_ps=uz_ps):
                    g0 = sg_ * SUPER + a_
                    a01 = ae_pool.tile([P, 8, P], bf16)
                    nc.vector.tensor_tensor(
                        out=a01[:, 0:gn_, :],
                        in0=iota8[:, 0:gn_, :],
                        in1=rel_sb[:, g0 : g0 + gn_].to_broadcast([P, gn_, P]),
                        op=ALU.is_equal,
                    )
                    aeb = ae_pool.tile([P, 8, P], bf16)
                    nc.vector.tensor_tensor(
                        out=aeb[:, 0:gn_, :],
                        in0=a01[:, 0:gn_, :],
                        in1=e_sb_[:, 0:gn_].to_broadcast([P, gn_, P]),
                        op=ALU.mult,
                    )
                    for t in range(gn_):
                        slot = a_ + t
                        ti = g0 + t - w * TPW  # tile index within window
                        nc.tensor.matmul(
                            out=uz_ps[:],
                            lhsT=aeb[:, t, :],
                            rhs=sup_t[:, slot, :],
                            start=(ti == 0),
                            stop=(ti == TPW - 1),
                        )

                pending = None
                for si in range(SUPERS_PER_WIN):
                    sg = w * SUPERS_PER_WIN + si
                    sup = xpool.tile([P, SUPER, D + 2], bf16)
                    nc.sync.dma_start(out=sup[:, :, 0:D], in_=x_r[sg])
                    nc.gpsimd.tensor_copy(
                        out=sup[:, :, D : D + 2],
                        in_=ones_sb[:].to_broadcast([P, SUPER, 2]),
                    )
                    sxt = xt_pool.tile([P, 2, SUPER * P], bf16)
                    n0 = sg * SUPER * P
                    nc.sync.dma_start(
                        out=sxt[:, 0, :], in_=xt_r[0][:, n0 : n0 + SUPER * P]
                    )
                    nc.sync.dma_start(
                        out=sxt[:, 1, :], in_=xt_r[1][:, n0 : n0 + SUPER * P]
                    )

                    for a, gn in gs:
                        # h = tanh(x @ W1 + b1): [hid, gn*128] in one PSUM
                        # tile spanning two banks; matmuls are k-ordered so
                        # W1 is loaded once per chunk per megagroup.
                        h_ps = ps_h.tile([P, 8 * P], f32)
                        halves = [(0, min(gn, 4))]
                        if gn > 4:
                            halves.append((4, gn - 4))
                        for k in range(2):
                            for ha, hn in halves:
                                nc.tensor.matmul(
                                    out=h_ps[:, ha * P : (ha + hn) * P],
                                    lhsT=w1_sb[:, k, :],
                                    rhs=sxt[:, k, (a + ha) * P : (a + ha + hn) * P],
                                    start=(k == 0),
                                    stop=(k == 1),
                                )
                        hb = hb_pool.tile([P, 8 * P], bf16)
                        nc.scalar.activation(
                            out=hb[:, 0 : gn * P],
                            in_=h_ps[:, 0 : gn * P],
                            func=AF.Tanh,
                            bias=b1_sb[:],
                            scale=1.0,
                        )
                        s_ps = ps_s.tile([P, 8, 2], f32)
                        for t in range(gn):
                            nc.tensor.matmul(
                                out=s_ps[:, t, :],
                                lhsT=hb[:, t * P : (t + 1) * P],
                                rhs=w2_sb[:],
                                start=True,
                                stop=True,
                            )
                        e_sb = e_pool.tile([P, 8], bf16)
                        nc.scalar.activation(
                            out=e_sb[:, 0:gn],
                            in_=s_ps[:, 0:gn, 0],
                            func=AF.Exp,
                            bias=b2_sb[:],
                            scale=1.0,
                        )
                        if pending is not None:
                            emit_ae(*pending)
                        pending = (sup, sg, a, gn, e_sb)
                emit_ae(*pending)

                # ---- flush window w
                uz_sb = flush_pool.tile([P, D + 1], f32)
                nc.vector.tensor_copy(out=uz_sb[:], in_=uz_ps[:, 0 : D + 1])
                nc.vector.tensor_add(
                    out=uz_sb[0:1, :], in0=uz_sb[0:1, :], in1=carry_sb[:]
                )
                c_ps = ps_c.tile([1, D + 1], f32)
                nc.tensor.matmul(
                    out=c_ps[:],
                    lhsT=oh_sb[:, w : w + 1],
                    rhs=uz_sb[:],
                    start=True,
                    stop=True,
                )
                nc.vector.tensor_copy(out=carry_sb[:], in_=c_ps[:])
                recip = flush_pool.tile([P, 1], f32)
                nc.vector.tensor_scalar_add(
                    out=recip[:], in0=uz_sb[:, D : D + 1], scalar1=EPS
                )
                nc.vector.reciprocal(out=recip[:], in_=recip[:])
                outw = flush_pool.tile([P, D], f32)
                nc.vector.tensor_scalar_mul(
                    out=outw[:], in0=uz_sb[:, 0:D], scalar1=recip[:]
                )
                nc.sync.dma_start(out=out_d[w * P : (w + 1) * P, :], in_=outw[:])

    nc.finalize()
    _NC_CACHE["nc"] = nc
    return nc


def _run(inputs, trace=False):
    from concourse.bass_utils import run_bass_kernel_spmd

    x = inputs["x"]
    plans = _plan(inputs["batch"])
    in_maps = _make_in_maps(
        x, inputs["W1"], inputs["b1"], inputs["W2"], inputs["b2"], plans
    )
    nc = _build_nc()
    res = run_bass_kernel_spmd(
        nc, in_maps, core_ids=list(range(NCORES)), trace=trace
    )
    outs = [r["out"] for r in res.results]
    final = _assemble(outs, plans, np.float32)
    return final, res


def kernel(**inputs):
    return _run(inputs, trace=False)[0]
